# revision 14
# baseline (speedup 1.0000x reference)
"""GraphSENN pool kernel for 8 trn2 NeuronCores (Bass/Tile).

Reference computation:
    h     = MLP3(x)                    # [N,10]  (relu, relu, linear)
    pooled= segment_sum(x, batch, 512) # [512,256]
    theta = MLP2(concat(x, pooled[batch]))  # [N,10]
    out   = segment_sum(h*theta, batch)     # [512,10]
    return (out, theta, h)

Strategy: data-parallel over nodes, 8 cores. batch is SORTED, so each core
touches a window of <=128 consecutive segment ids. Segment scatter/gather are
one-hot matmuls on PE (one-hot built by DVE is_equal against an iota row).
pooled@W_b is projected locally, scattered to global coords via a one-hot
matmul, AllReduced on-device, and the local window selected back. Activations
ride feature-major (weights stationary as lhsT) so bias+relu fuse on ScalarE.
All matmul operands are bf16 (fp32 PSUM accumulation): fp32 matmul is
quarter-rate on the trn2 PE. Host supplies x in both layouts (feature-major
for the MLPs, node-major for the pooled accumulation) so the kernel does no
x transposes. One PSUM bank per independent accumulation (start=True clears
the whole bank).
"""

import numpy as np

# ---- problem constants (hardcoded; kernel.py must be self-contained) ----
N = 300000
D = 256
C = 10
G = 512
P = 8                  # cores
NPC = N // P           # 37500 nodes per core
ST = 512               # nodes per supertile
NT = 128               # nodes per node-tile
LW = 128               # local segment window width
NST = 74               # supertiles per core
NLOC = NST * ST        # 37888 padded nodes per core
NTILES = NLOC // NT    # 296 node-tiles per core
PAD_SEG = 999.0        # relative seg id for padding rows (never matches)


def _f32(a):
    return np.ascontiguousarray(a, dtype=np.float32)


def _bf16(a):
    import ml_dtypes
    return np.ascontiguousarray(np.asarray(a, np.float32).astype(ml_dtypes.bfloat16))


def _build_program(w, use_collective=True):
    """Build the SPMD Bass program. `w` holds the weight arrays."""
    import concourse.bacc as bacc
    import concourse.mybir as mybir
    import concourse.tile as tile

    f32 = mybir.dt.float32
    bf16 = mybir.dt.bfloat16
    AF = mybir.ActivationFunctionType
    OP = mybir.AluOpType

    nc = bacc.Bacc("TRN2", target_bir_lowering=False, debug=False, num_devices=P)

    # ---- kernel I/O ----
    xT_d = nc.dram_tensor("xT", [2, 128, NLOC], bf16, kind="ExternalInput")
    xnm_d = nc.dram_tensor("xnm", [NLOC, D], bf16, kind="ExternalInput")
    bt_d = nc.dram_tensor("bt", [128, NTILES], f32, kind="ExternalInput")
    glo_d = nc.dram_tensor("glo", [128, 1], f32, kind="ExternalInput")
    hT_d = nc.dram_tensor("hT", [C, NLOC], f32, kind="ExternalOutput")
    thT_d = nc.dram_tensor("thT", [C, NLOC], f32, kind="ExternalOutput")
    outw_d = nc.dram_tensor("outw", [LW, C], f32, kind="ExternalOutput")

    # ---- inline constants (baked into NEFF), pre-shaped for SBUF ----
    def blk22(W):  # [256,256] -> [128, 4*128], col block b=(k*2+m) holds W[k][m]
        return W.reshape(2, 128, 2, 128).transpose(1, 0, 2, 3).reshape(128, 512)

    def blkk(W):   # [256,C] -> [128, 2*C], col block k holds W[k]
        return W.reshape(2, 128, C).transpose(1, 0, 2).reshape(128, 2 * C)

    consts = {
        "iota_lw": _f32(np.tile(np.arange(LW), (128, 1))),
        "iota512": _f32(np.tile(np.arange(G), (128, 1))),
        "iota_col": _f32(np.arange(128).reshape(128, 1)),
        "ident": _f32(np.eye(128)),
        "ident_bf": _bf16(np.eye(128)),
        "hW0": _bf16(blk22(w["h_W0"])),
        "hW1": _bf16(blk22(w["h_W1"])),
        "hW2": _bf16(blkk(w["h_W2"])),
        "tA": _bf16(blk22(w["t_W0"][:D])),
        "tB": _f32(blk22(w["t_W0"][D:])),
        "tW1": _bf16(blkk(w["t_W1"])),
        "hb0": _f32(w["h_b0"].reshape(2, 128).T),
        "hb1": _f32(w["h_b1"].reshape(2, 128).T),
        "hb2": _f32(w["h_b2"].reshape(C, 1)),
        "tb0": _f32(w["t_b0"].reshape(2, 128).T),
        "tb1": _f32(w["t_b1"].reshape(C, 1)),
    }
    cdram = {k: nc.inline_tensor(v, name=f"c_{k}") for k, v in consts.items()}
    cdt = {k: (f32 if v.dtype == np.float32 else bf16) for k, v in consts.items()}

    def W22(t, k, m):
        b = k * 2 + m
        return t[:, b * 128:(b + 1) * 128]

    def WKC(t, k):
        return t[:, k * C:(k + 1) * C]

    with tile.TileContext(nc) as tc:
        with (
            tc.tile_pool(name="const", bufs=1) as cp,
            tc.tile_pool(name="dram", bufs=1, space="DRAM") as dp,
        ):
            def ctile(key):
                a = consts[key]
                t = cp.tile(list(a.shape), cdt[key], tag=f"c_{key}")
                nc.sync.dma_start(t[:, :], cdram[key][:, :])
                return t

            sb = {k: ctile(k) for k in consts}
            bt_sb = cp.tile([128, NTILES], f32, tag="bt")
            nc.sync.dma_start(bt_sb[:, :], bt_d[:, :])
            glo_sb = cp.tile([128, 1], f32, tag="glo")
            nc.sync.dma_start(glo_sb[:, :], glo_d[:, :])

            h_stash = dp.tile([C, NLOC], f32, tag="h_stash")
            ar_in = dp.tile([G, D], f32, tag="ar_in")
            ar_out = dp.tile([G, D], f32, tag="ar_out")

            IOTA_LW = sb["iota_lw"]
            IOTA512 = sb["iota512"]
            IOTA_COL = sb["iota_col"]
            IDENT = sb["ident"]
            IDENT_BF = sb["ident_bf"]

            with tc.tile_pool(name="pooled_ps", bufs=1, space="PSUM") as plp:
                # seg-major pooled accumulator [LW, 256] (half a bank)
                pooled_ps = plp.tile([LW, D], f32, tag="pooled")

                # ============ PHASE 1: h MLP + pooled accumulation ============
                with (
                    tc.tile_pool(name="p1_x", bufs=6) as xp,
                    tc.tile_pool(name="p1_act", bufs=6) as ap,
                    tc.tile_pool(name="p1_small", bufs=12) as sp,
                    tc.tile_pool(name="p1_mlp_ps", bufs=5, space="PSUM") as mp,
                    tc.tile_pool(name="p1_h3_ps", bufs=2, space="PSUM") as hp3,
                ):
                    for st in range(NST):
                        sl = slice(st * ST, (st + 1) * ST)
                        xt = []
                        for k in range(2):
                            t = xp.tile([128, ST], bf16, tag=f"xt{k}")
                            nc.sync.dma_start(t[:, :], xT_d[k, :, sl])
                            xt.append(t)
                        # h1 = relu(W0.T x + b0), feature-major, bf16 out
                        h1 = []
                        for m in range(2):
                            ps = mp.tile([128, ST], f32, tag="mlp")
                            nc.tensor.matmul(ps[:, :], W22(sb["hW0"], 0, m),
                                             xt[0][:, :], start=True, stop=False)
                            nc.tensor.matmul(ps[:, :], W22(sb["hW0"], 1, m),
                                             xt[1][:, :], start=False, stop=True)
                            o = ap.tile([128, ST], bf16, tag=f"h1_{m}")
                            nc.scalar.activation(o[:, :], ps[:, :], AF.Relu,
                                                 bias=sb["hb0"][:, m:m + 1])
                            h1.append(o)
                        h2 = []
                        for m in range(2):
                            ps = mp.tile([128, ST], f32, tag="mlp")
                            nc.tensor.matmul(ps[:, :], W22(sb["hW1"], 0, m),
                                             h1[0][:, :], start=True, stop=False)
                            nc.tensor.matmul(ps[:, :], W22(sb["hW1"], 1, m),
                                             h1[1][:, :], start=False, stop=True)
                            o = ap.tile([128, ST], bf16, tag=f"h2_{m}")
                            nc.scalar.activation(o[:, :], ps[:, :], AF.Relu,
                                                 bias=sb["hb1"][:, m:m + 1])
                            h2.append(o)
                        ps3 = hp3.tile([C, ST], f32, tag="h3")
                        nc.tensor.matmul(ps3[:, :], WKC(sb["hW2"], 0), h2[0][:, :],
                                         start=True, stop=False)
                        nc.tensor.matmul(ps3[:, :], WKC(sb["hW2"], 1), h2[1][:, :],
                                         start=False, stop=True)
                        hts = ap.tile([C, ST], f32, tag="hts")
                        nc.vector.tensor_scalar(hts[:, :], ps3[:, :],
                                                sb["hb2"][:, :], None, op0=OP.add)
                        nc.sync.dma_start(hT_d[:, sl], hts[:, :])
                        nc.sync.dma_start(h_stash[:, sl], hts[:, :])

                        # pooled_sm[l, f] += sum_n S[n,l] * x[n,f]
                        xn = sp.tile([128, 4, D], bf16, tag="xn")
                        nc.sync.dma_start(
                            xn[:, :, :],
                            xnm_d[sl, :].rearrange("(t p) d -> p t d", p=NT))
                        for nt in range(4):
                            t_idx = st * 4 + nt
                            S = sp.tile([128, LW], bf16, tag="S")
                            nc.vector.tensor_scalar(
                                S[:, :], IOTA_LW[:, :], bt_sb[:, t_idx:t_idx + 1],
                                None, op0=OP.is_equal)
                            nc.tensor.matmul(
                                pooled_ps[:, :], S[:, :], xn[:, nt, :],
                                start=(t_idx == 0), stop=(t_idx == NTILES - 1),
                                skip_group_check=True)

                # ============ MID: project, scatter, AllReduce, select ========
                with (
                    tc.tile_pool(name="mid_sb", bufs=1) as msb,
                    tc.tile_pool(name="mid_ps", bufs=2, space="PSUM") as mps,
                ):
                    pooled_sm = msb.tile([LW, D], f32, tag="pooled_sm")
                    nc.vector.tensor_copy(pooled_sm[:, :], pooled_ps[:, :])
                    # transpose to feature-major pooledT chunks [128, LW]
                    pooled_fm = msb.tile([128, D], f32, tag="pooled_fm")
                    for k in range(2):
                        ps = mps.tile([128, 256], f32, tag="mid")
                        nc.tensor.transpose(ps[:, 0:LW],
                                            pooled_sm[:, k * 128:(k + 1) * 128],
                                            IDENT[:, :])
                        nc.vector.tensor_copy(pooled_fm[:, k * 128:(k + 1) * 128],
                                              ps[:, 0:LW])
                    # ppT[m] = sum_k tB[k,m].T @ pooledT[k], then transpose to
                    # seg-major pp_sm [LW, 256]
                    pp_sm = msb.tile([LW, D], f32, tag="pp_sm")
                    for m in range(2):
                        ps = mps.tile([128, 256], f32, tag="mid")
                        nc.tensor.matmul(ps[:, 0:LW], W22(sb["tB"], 0, m),
                                         pooled_fm[:, 0:128],
                                         start=True, stop=False)
                        nc.tensor.matmul(ps[:, 0:LW], W22(sb["tB"], 1, m),
                                         pooled_fm[:, 128:256],
                                         start=False, stop=True)
                        ppT_sb = msb.tile([128, LW], f32, tag=f"ppT_{m}")
                        nc.vector.tensor_copy(ppT_sb[:, :], ps[:, 0:LW])
                        ps2 = mps.tile([128, 256], f32, tag="mid")
                        nc.tensor.transpose(ps2[:, 0:128], ppT_sb[:, :],
                                            IDENT[:, :])
                        nc.vector.tensor_copy(pp_sm[:, m * 128:(m + 1) * 128],
                                              ps2[:, 0:128])
                    # L2G one-hot [LW, G]: L2G[l, g] = (g == g_lo + l)
                    labs = msb.tile([128, 1], f32, tag="labs")
                    nc.vector.tensor_tensor(labs[:, :], IOTA_COL[:, :],
                                            glo_sb[:, :], op=OP.add)
                    L2G = msb.tile([128, G], f32, tag="L2G")
                    nc.vector.tensor_scalar(L2G[:, :], IOTA512[:, :], labs[:, :],
                                            None, op0=OP.is_equal)
                    for cg in range(4):
                        ps = mps.tile([128, 256], f32, tag="mid")
                        nc.tensor.matmul(ps[:, :],
                                         L2G[:, cg * 128:(cg + 1) * 128],
                                         pp_sm[:, :], start=True, stop=True)
                        scs = msb.tile([128, 256], f32, tag=f"sc_{cg}")
                        nc.vector.tensor_copy(scs[:, :], ps[:, :])
                        nc.gpsimd.dma_start(ar_in[cg * 128:(cg + 1) * 128, :],
                                            scs[:, :])
                    if use_collective:
                        nc.gpsimd.collective_compute(
                            "AllReduce", OP.add,
                            replica_groups=[list(range(P))],
                            ins=[ar_in[:, :].opt()],
                            outs=[ar_out[:, :].opt()],
                        )
                    else:  # timing-sim variant: AR ~= DRAM copy + ~17us
                        nc.gpsimd.dma_start(ar_out[:, :], ar_in[:, :])
                    ppw_ps = mps.tile([128, 256], f32, tag="midw")
                    for cg in range(4):
                        ars = msb.tile([128, 256], f32, tag=f"ar_{cg}")
                        nc.gpsimd.dma_start(ars[:, :],
                                            ar_out[cg * 128:(cg + 1) * 128, :])
                        tmp = msb.tile([128, 1], f32, tag=f"g2l_t_{cg}")
                        nc.vector.tensor_scalar(tmp[:, :], IOTA_COL[:, :],
                                                float(cg * 128), None, op0=OP.add)
                        tmp2 = msb.tile([128, 1], f32, tag=f"g2l_u_{cg}")
                        nc.vector.tensor_tensor(tmp2[:, :], tmp[:, :],
                                                glo_sb[:, :], op=OP.subtract)
                        G2L = msb.tile([128, LW], f32, tag=f"G2L_{cg}")
                        nc.vector.tensor_scalar(G2L[:, :], IOTA_LW[:, :],
                                                tmp2[:, :], None, op0=OP.is_equal)
                        nc.tensor.matmul(ppw_ps[:, :], G2L[:, :], ars[:, :],
                                         start=(cg == 0), stop=(cg == 3),
                                         skip_group_check=True)
                    ppw_sb = cp.tile([LW, 256], bf16, tag="ppw_sb")
                    nc.vector.tensor_copy(ppw_sb[:, :], ppw_ps[:, :])

            # ============ PHASE 2: theta MLP + output accumulation ============
            with (
                tc.tile_pool(name="p2_x", bufs=6) as xp2,
                tc.tile_pool(name="p2_act", bufs=6) as ap2,
                tc.tile_pool(name="p2_small", bufs=14) as sp2,
                tc.tile_pool(name="p2_out_ps", bufs=1, space="PSUM") as op2,
                tc.tile_pool(name="p2_t1_ps", bufs=3, space="PSUM") as t1p,
                tc.tile_pool(name="p2_t2_ps", bufs=2, space="PSUM") as t2p,
                tc.tile_pool(name="p2_tp_ps", bufs=2, space="PSUM") as tp2,
            ):
                out_ps = op2.tile([LW, C], f32, tag="out_ps")
                for st in range(NST):
                    sl = slice(st * ST, (st + 1) * ST)
                    xt = []
                    for k in range(2):
                        t = xp2.tile([128, ST], bf16, tag=f"xt{k}")
                        nc.sync.dma_start(t[:, :], xT_d[k, :, sl])
                        xt.append(t)
                    hts = ap2.tile([C, ST], f32, tag="hts")
                    nc.sync.dma_start(hts[:, :], h_stash[:, sl])
                    SLT = ap2.tile([LW, ST], bf16, tag="SLT")
                    S_nt = []
                    for nt in range(4):
                        t_idx = st * 4 + nt
                        S = sp2.tile([128, LW], bf16, tag="S2")
                        nc.vector.tensor_scalar(
                            S[:, :], IOTA_LW[:, :], bt_sb[:, t_idx:t_idx + 1],
                            None, op0=OP.is_equal)
                        S_nt.append(S)
                        tp = tp2.tile([128, 128], bf16, tag="tp2")
                        nc.tensor.transpose(tp[:, :], S[:, :], IDENT_BF[:, :])
                        nc.vector.tensor_copy(SLT[:, nt * NT:(nt + 1) * NT],
                                              tp[:, :])
                    t1 = []
                    for m in range(2):
                        ps = t1p.tile([128, ST], f32, tag="t1")
                        nc.tensor.matmul(ps[:, :], W22(sb["tA"], 0, m),
                                         xt[0][:, :], start=True, stop=False)
                        nc.tensor.matmul(ps[:, :], W22(sb["tA"], 1, m),
                                         xt[1][:, :], start=False, stop=False)
                        nc.tensor.matmul(ps[:, :],
                                         ppw_sb[:, m * 128:(m + 1) * 128],
                                         SLT[:, :], start=False, stop=True)
                        o = ap2.tile([128, ST], bf16, tag=f"t1_{m}")
                        nc.scalar.activation(o[:, :], ps[:, :], AF.Relu,
                                             bias=sb["tb0"][:, m:m + 1])
                        t1.append(o)
                    ps2 = t2p.tile([C, ST], f32, tag="t2")
                    nc.tensor.matmul(ps2[:, :], WKC(sb["tW1"], 0), t1[0][:, :],
                                     start=True, stop=False)
                    nc.tensor.matmul(ps2[:, :], WKC(sb["tW1"], 1), t1[1][:, :],
                                     start=False, stop=True)
                    ths = ap2.tile([C, ST], f32, tag="ths")
                    nc.vector.tensor_scalar(ths[:, :], ps2[:, :],
                                            sb["tb1"][:, :], None, op0=OP.add)
                    nc.sync.dma_start(thT_d[:, sl], ths[:, :])
                    hth = ap2.tile([C, ST], bf16, tag="hth")
                    nc.vector.tensor_tensor(hth[:, :], ths[:, :], hts[:, :],
                                            op=OP.mult)
                    for nt in range(4):
                        tp = tp2.tile([128, 128], bf16, tag="tp2")
                        nc.tensor.transpose(tp[:128, :C],
                                            hth[:, nt * NT:(nt + 1) * NT],
                                            IDENT_BF[:C, :C])
                        htnm = sp2.tile([128, C], bf16, tag="htnm")
                        nc.vector.tensor_copy(htnm[:, :], tp[:128, :C])
                        nc.tensor.matmul(out_ps[:, :], S_nt[nt][:, :],
                                         htnm[:, :],
                                         start=(st == 0 and nt == 0),
                                         stop=(st == NST - 1 and nt == 3),
                                         skip_group_check=True)
                outw_sb = cp.tile([LW, C], f32, tag="outw_sb")
                nc.vector.tensor_copy(outw_sb[:, :], out_ps[:, :])
                nc.sync.dma_start(outw_d[:, :], outw_sb[:, :])

    nc.compile()
    return nc


def _prep_inputs(x, batch):
    """Shard + pad + transpose on host. Returns (in_maps, g_los)."""
    import ml_dtypes
    bf = ml_dtypes.bfloat16
    x = np.asarray(x, dtype=np.float32)
    batch = np.asarray(batch).astype(np.int64)
    in_maps, g_los = [], []
    for c in range(P):
        lo, hi = c * NPC, (c + 1) * NPC
        xb = x[lo:hi].astype(bf)
        bb = batch[lo:hi]
        g_lo = int(bb[0])
        span = int(bb[-1]) - g_lo + 1
        assert span <= LW, f"core {c}: segment span {span} > {LW}"
        xT = np.zeros((2, 128, NLOC), dtype=bf)
        xT[:, :, :NPC] = xb.T.reshape(2, 128, NPC)
        xnm = np.zeros((NLOC, D), dtype=bf)
        xnm[:NPC] = xb
        brel = np.full(NLOC, PAD_SEG, dtype=np.float32)
        brel[:NPC] = (bb - g_lo).astype(np.float32)
        bt = np.ascontiguousarray(brel.reshape(NTILES, 128).T)
        glo = np.full((128, 1), float(g_lo), dtype=np.float32)
        in_maps.append({"xT": xT, "xnm": xnm, "bt": bt, "glo": glo})
        g_los.append(g_lo)
    return in_maps, g_los


def _postprocess(results, g_los):
    h = np.concatenate([r["hT"][:, :NPC].T for r in results], axis=0)
    theta = np.concatenate([r["thT"][:, :NPC].T for r in results], axis=0)
    out = np.zeros((G, C), dtype=np.float32)
    for c in range(P):
        g_lo = g_los[c]
        k = min(LW, G - g_lo)
        out[g_lo:g_lo + k] += results[c]["outw"][:k]
    return out, theta, h


def _run(inputs, trace=False):
    from concourse.bass_utils import run_bass_kernel_spmd

    w = {k: np.asarray(v, dtype=np.float32) for k, v in inputs.items()
         if k not in ("x", "batch")}
    nc = _build_program(w)
    in_maps, g_los = _prep_inputs(inputs["x"], inputs["batch"])
    res = run_bass_kernel_spmd(nc, in_maps, list(range(P)), trace=trace)
    out, theta, h = _postprocess(res.results, g_los)
    return (out, theta, h), res


def kernel(**inputs):
    (out, theta, h), _ = _run(inputs, trace=False)
    return (out, theta, h)


def kernel_traced(**inputs):
    (out, theta, h), res = _run(inputs, trace=True)
    return (out, theta, h), res


def _run_timed(inputs, iters=6):
    """Run with inputs pre-placed on device; time repeated executions."""
    import time
    import jax
    from jax.sharding import Mesh, PartitionSpec, NamedSharding
    from jax.experimental.shard_map import shard_map
    from concourse import bass2jax
    from concourse import mybir

    w = {k: np.asarray(v, dtype=np.float32) for k, v in inputs.items()
         if k not in ("x", "batch")}
    nc = _build_program(w)
    in_maps, g_los = _prep_inputs(inputs["x"], inputs["batch"])

    bass2jax.install_neuronx_cc_hook()
    partition_name = (nc.partition_id_tensor.name
                      if nc.partition_id_tensor else None)
    in_names, out_names, out_avals, zero_outs = [], [], [], []
    for alloc in nc.m.functions[0].allocations:
        if not isinstance(alloc, mybir.MemoryLocationSet):
            continue
        name = alloc.memorylocations[0].name
        if alloc.kind == "ExternalInput":
            if name != partition_name:
                in_names.append(name)
        elif alloc.kind == "ExternalOutput":
            shape = tuple(alloc.tensor_shape)
            dtype = mybir.dt.np(alloc.dtype)
            out_names.append(name)
            out_avals.append(jax.core.ShapedArray(shape, dtype))
            zero_outs.append(np.zeros(shape, dtype))
    n_params = len(in_names)
    n_outs = len(out_avals)
    all_in_names = list(in_names) + list(out_names)
    if partition_name is not None:
        all_in_names.append(partition_name)
    donate = tuple(range(n_params, n_params + n_outs))

    def _body(*args):
        operands = list(args)
        if partition_name is not None:
            operands.append(bass2jax.partition_id_tensor())
        outs = bass2jax._bass_exec_p.bind(
            *operands,
            out_avals=tuple(out_avals),
            in_names=tuple(all_in_names),
            out_names=tuple(out_names),
            lowering_input_output_aliases=(),
            sim_require_finite=True,
            sim_require_nnan=True,
            nc=nc,
        )
        return tuple(outs)

    devices = jax.devices()[:P]
    mesh = Mesh(np.asarray(devices), ("core",))
    spec = PartitionSpec("core")
    sharded = jax.jit(
        shard_map(_body, mesh=mesh, in_specs=(spec,) * (n_params + n_outs),
                  out_specs=(spec,) * n_outs, check_rep=False),
        donate_argnums=donate, keep_unused=True,
    )
    shd = NamedSharding(mesh, spec)
    concat_in = [
        jax.device_put(
            np.concatenate([np.asarray(in_maps[c][nm]) for c in range(P)], 0),
            shd)
        for nm in in_names
    ]

    def fresh_zeros():
        return [jax.device_put(
                    np.zeros((P * z.shape[0], *z.shape[1:]), z.dtype), shd)
                for z in zero_outs]

    out_arrs = sharded(*concat_in, *fresh_zeros())
    jax.block_until_ready(out_arrs)
    results = [
        {nm: np.asarray(out_arrs[i]).reshape(P, *out_avals[i].shape)[c]
         for i, nm in enumerate(out_names)}
        for c in range(P)
    ]
    times = []
    for _ in range(iters):
        zs = fresh_zeros()
        jax.block_until_ready(zs)
        t0 = time.perf_counter()
        o = sharded(*concat_in, *zs)
        jax.block_until_ready(o)
        times.append(time.perf_counter() - t0)
    out, theta, h = _postprocess(results, g_los)
    return (out, theta, h), times
